# revision 34
# baseline (speedup 1.0000x reference)
"""Trainium2 Bass kernel for nn_AttentionModule (segment_reduce).

Computation (per reference):
    wx   = features @ W
    s_g  = segment_sum(wx);  cnt_g = segment counts
    ctx  = tanh(s_g / max(cnt,1))            [G, D]
    score_n = <f_n, ctx[seg_n]>
    rep_g = segment_sum(score_n * f_n)       [G, D]

Key identity: segment_sum(F @ W) = segment_sum(F) @ W, so pass A is a pure
segment-sum.  Device algorithm (SPMD over 8 cores, segment-aligned shard
per core, one uniform compiled program; all data-dependence flows through
input tensors, never instruction addresses):

  per 2048-node chunk (16 new 128-node tiles + 1 boundary re-read tile):
    - one-hot (node x 32-wide pair window) built on DVE by comparing an
      iota const against host-provided window-relative ids
    - segment-sum via PE matmuls into 32-aligned psum slot ranges;
      slot->window merge matmul with an on-device generated 0/1 matrix
    - ctx = tanh((sum @ W) * -recip) (negated; compensates the negated
      transposed one-hot below)
    - transposed one-hot ohT[w, n] = -(relrow[n]==w) via a K=2 broadcast
      matmul (diff = relrow[n] - w), ACT Square, GPSIMD min(d^2-1, 0)
    - CtxG = ohT.T @ (-ctx) (PE), scores = rowsum(F * CtxG) (DVE),
      rep via one-hot matmul + merge, written chunk-major to DRAM
  host: numpy pre/post-processing (index metadata, shard assembly).
"""

import os
import sys
import math
from functools import lru_cache

for _p in ("/opt/trn_rl_repo", "/root/.axon_site/_ro/trn_rl_repo"):
    if os.path.isdir(_p) and _p not in sys.path:
        sys.path.insert(0, _p)

import numpy as np
import ml_dtypes

BF16 = ml_dtypes.bfloat16

P = 128          # partitions
TPC = 16         # new 128-node tiles per chunk
CHUNK = TPC * P  # 2048 new nodes per chunk
TILES = TPC + 1  # + boundary tile (tile 0)
NODES = TILES * P  # nodes touched per chunk (2176)
WINW = 128       # chunk segment-window width
PAIRW = 32       # pair segment-window width
NCORES = 8
MASK = -1000.0


def _ensure_ntff_hook():
    """Register the axon NTFF profile hook if the boot couldn't (antenv stub)."""
    import types
    try:
        import antenv  # noqa
    except ImportError:
        return
    if "antenv.axon_hooks" in sys.modules:
        return
    hooks = types.ModuleType("antenv.axon_hooks")
    holder = [None]
    hooks.set_axon_ntff_profile_hook = lambda h: holder.__setitem__(0, h)
    hooks.get_axon_ntff_profile_hook = lambda: holder[0]
    sys.modules["antenv.axon_hooks"] = hooks
    import antenv
    antenv.axon_hooks = hooks
    try:
        from trn_agent_boot.trn_boot import _ntff_profile_via_ctypes
        so = "/opt/axon/libaxon_pjrt.so"
        if os.path.exists(so):
            hooks.set_axon_ntff_profile_hook(_ntff_profile_via_ctypes(so))
    except Exception:
        pass


@lru_cache(maxsize=4)
def build_program(nch: int, tpc: int = TPC):
    """Build + compile the uniform per-core Bass program.

    Returns (nc, io_names) where io_names lists the dram tensor names.
    """
    import concourse.bass as bass  # noqa
    import concourse.mybir as mybir
    from concourse import bacc, tile

    tiles = tpc + 1
    nodes = tiles * P
    n_pairs = tpc // 2
    n_pt = (n_pairs + 3) // 4          # psum partial tiles (4 pairs each)
    n_g4 = (tiles + 3) // 4            # ctx-gather groups of 4 tiles
    tq = tpc // 4                      # tiles per quarter-window
    f32 = mybir.dt.float32
    bf16 = mybir.dt.bfloat16

    nc = bacc.Bacc("TRN2", target_bir_lowering=False, debug=False,
                   num_devices=NCORES)

    # ---- DRAM I/O ----
    F_d = nc.dram_tensor("f_in", [nch, P, tiles, P], bf16,
                         kind="ExternalInput").ap()
    oh01_d = nc.dram_tensor("oh01", [nch, P, tiles, PAIRW], bf16,
                            kind="ExternalInput").ap()
    relrow_d = nc.dram_tensor("relrow", [nch, 1, nodes], bf16,
                              kind="ExternalInput").ap()
    m01_d = nc.dram_tensor("m01", [nch, P, n_pt, WINW], bf16,
                           kind="ExternalInput").ap()
    piota_d = nc.dram_tensor("piota", [P, 1], mybir.dt.float32,
                             kind="ExternalInput").ap()
    W_d = nc.dram_tensor("w_in", [P, P], bf16, kind="ExternalInput").ap()
    rep_d = nc.dram_tensor("rep_out", [nch, P, n_pt, P], bf16,
                           kind="ExternalOutput").ap()

    AluOp = mybir.AluOpType
    Act = mybir.ActivationFunctionType

    with tile.TileContext(nc) as tc:
        with tc.tile_pool(name="const", bufs=1) as cpool, \
             tc.tile_pool(name="fpool", bufs=3) as fpool, \
             tc.tile_pool(name="small", bufs=3) as spool, \
             tc.tile_pool(name="big", bufs=3) as bpool, \
             tc.tile_pool(name="ps_ctx", bufs=1, space="PSUM") as pss, \
             tc.tile_pool(name="ps_acc", bufs=2, space="PSUM") as psa, \
             tc.tile_pool(name="ps_big", bufs=3, space="PSUM") as psb:

            piota_t = cpool.tile([P, 1], mybir.dt.float32)
            w_t = cpool.tile([P, P], bf16)
            nc.sync.dma_start(piota_t[:], piota_d[:])
            nc.sync.dma_start(w_t[:], W_d[:])

            for k in range(nch):
                # ---- loads ----
                f_t = fpool.tile([P, tiles, P], bf16, tag="f")
                nc.sync.dma_start(f_t[:], F_d[k])

                m01_t = spool.tile([P, n_pt, WINW], bf16, tag="m01")
                nc.sync.dma_start(m01_t[:], m01_d[k])

                # ---- one-hot [node, pair-window] (host-built) ----
                oh_t = spool.tile([P, tiles, PAIRW], bf16, tag="oh")
                nc.sync.dma_start(oh_t[:], oh01_d[k])


                def tile_order():
                    # first MM into each psum partial tile must be a fresh
                    # (start=True) write; boundary tile 0 accumulates into
                    # tile 1's slot range so it must come after tile 1.
                    order = [1, 0] + list(range(2, tiles))
                    return order

                def slot_base(t):
                    u = (t - 1) // 2 if t >= 1 else 0
                    return (u % 4) * PAIRW, u // 4

                # ---- step1: segment sums into slots ----
                order = tile_order()
                # region (pair) u: writers in order; first gets start=True,
                # last gets stop=True.  boundary tile 0 shares region 0.
                region_of = {t: ((t - 1) // 2 if t >= 1 else 0) for t in order}
                first_of_region = {}
                last_of_region = {}
                for t in order:
                    u = region_of[t]
                    first_of_region.setdefault(u, t)
                    last_of_region[u] = t
                ps_s = psa.tile([P, n_pt, P], f32, tag="ps_s",
                                name=f"ps_s_{k}")
                for t in order:
                    base, pt = slot_base(t)
                    u = region_of[t]
                    nc.tensor.matmul(
                        ps_s[base:base + PAIRW, pt, :],
                        oh_t[:, t, :], f_t[:, t, :],
                        start=(first_of_region[u] == t),
                        stop=(last_of_region[u] == t),
                        tile_position=(0, base))

                # ---- merge to sumT [D, w] ----
                ps_sumT = pss.tile([P, WINW], f32, tag="ctxps")
                s_sb = spool.tile([P, n_pt, P], bf16, tag="s_sb")
                nc.scalar.copy(s_sb[:], ps_s[:])
                for pt in range(n_pt):
                    nc.tensor.matmul(ps_sumT[:], s_sb[:, pt, :], m01_t[:, pt, :],
                                     start=(pt == 0), stop=(pt == n_pt - 1))

                # ---- ctx: pre[q] = (recip*sum)[32q:32(q+1)] @ W, quarter-
                # aligned at partitions 0:32 so the gather contracts over
                # K=32 (recip is folded into m01 on the host) ----
                sumT_sb = spool.tile([P, WINW], bf16, tag="sumT_sb")
                nc.scalar.copy(sumT_sb[:], ps_sumT[:])
                ps_preq = pss.tile([PAIRW, 4, P], f32, tag="ctxps")
                for q in range(4):
                    nc.tensor.matmul(ps_preq[:, q, :],
                                     sumT_sb[:, q * PAIRW:(q + 1) * PAIRW],
                                     w_t[:], start=True, stop=True)
                ctxq_t = spool.tile([PAIRW, 4, P], bf16, tag="ctxq")
                nc.scalar.activation(ctxq_t[:], ps_preq[:], Act.Tanh)

                # ---- ohT[j, n] = (relrow[n] == j), quarter-local ----
                relb_t = bpool.tile([PAIRW, nodes], bf16, tag="relb")
                nc.sync.dma_start(relb_t[:],
                                  relrow_d[k].broadcast_to([PAIRW, nodes]))
                ohT_t = bpool.tile([PAIRW, nodes], bf16, tag="ohT")
                nc.vector.tensor_scalar(out=ohT_t[:], in0=relb_t[:],
                                        scalar1=piota_t[0:PAIRW, :],
                                        scalar2=None,
                                        op0=AluOp.is_equal)

                # ---- 3a: CtxG groups + scores ----
                def quarter_of(t):
                    return 0 if t == 0 else min((t - 1) // tq, 3)

                prod_sb = bpool.tile([P, tiles, P], bf16, tag="prod")
                n_act = 7          # gather groups routed psum->sbuf via ACT
                for g in range(n_g4):
                    t0g = g * 4
                    ng = min(4, tiles - t0g)
                    ps_cg = psb.tile([P, 512], f32, tag="bigps")
                    for i in range(ng):
                        t = t0g + i
                        nc.tensor.matmul(
                            ps_cg[:, i * P:(i + 1) * P],
                            ohT_t[:, t * P:(t + 1) * P],
                            ctxq_t[:, quarter_of(t), :],
                            start=(i == 0), stop=(i == ng - 1))
                    if g < n_act:
                        # ACT copies psum->sbuf bf16 so the DVE multiply
                        # runs in 2x mode (all-2-byte operands)
                        cg_sb = spool.tile([P, 512], bf16, tag="cg_sb")
                        nc.scalar.copy(cg_sb[:, :ng * P], ps_cg[:, :ng * P])
                        nc.vector.tensor_tensor(
                            out=prod_sb[:, t0g:t0g + ng, :],
                            in0=f_t[:, t0g:t0g + ng, :],
                            in1=cg_sb[:, :ng * P].rearrange(
                                "p (t d) -> p t d", d=P),
                            op=AluOp.mult)
                    else:
                        nc.vector.tensor_tensor(
                            out=prod_sb[:, t0g:t0g + ng, :],
                            in0=f_t[:, t0g:t0g + ng, :],
                            in1=ps_cg[:, :ng * P].rearrange(
                                "p (t d) -> p t d", d=P),
                            op=AluOp.mult)
                fold_sb = spool.tile([P, tiles, P // 2], bf16, tag="fold")
                nc.vector.tensor_tensor(
                    out=fold_sb[:], in0=prod_sb[:, :, :P // 2],
                    in1=prod_sb[:, :, P // 2:], op=AluOp.add)
                fold2_sb = spool.tile([P, tiles, P // 4], bf16, tag="fold2")
                nc.vector.tensor_tensor(
                    out=fold2_sb[:], in0=fold_sb[:, :, :P // 4],
                    in1=fold_sb[:, :, P // 4:], op=AluOp.add)
                scores_b = spool.tile([P, tiles], bf16, tag="scores_b")
                with nc.allow_low_precision(
                        reason="scores reduce: bf16 out is within budget"):
                    nc.vector.tensor_reduce(out=scores_b[:], in_=fold2_sb[:],
                                            axis=mybir.AxisListType.X,
                                            op=AluOp.add)

                # ---- 3b: rep sums ----
                ohsc_t = spool.tile([P, tiles, PAIRW], bf16, tag="ohsc")
                nc.vector.tensor_tensor(
                    out=ohsc_t[:], in0=oh_t[:],
                    in1=scores_b[:].unsqueeze(2).broadcast_to(
                        [P, tiles, PAIRW]),
                    op=AluOp.mult)
                ps_r = psa.tile([P, n_pt, P], f32, tag="ps_r",
                                name=f"ps_r_{k}")
                for t in order:
                    base, pt = slot_base(t)
                    u = region_of[t]
                    nc.tensor.matmul(
                        ps_r[base:base + PAIRW, pt, :],
                        ohsc_t[:, t, :], f_t[:, t, :],
                        start=(first_of_region[u] == t),
                        stop=(last_of_region[u] == t),
                        tile_position=(0, base))
                r_sb = spool.tile([P, n_pt, P], bf16, tag="r_sb")
                nc.scalar.copy(r_sb[:], ps_r[:])
                nc.sync.dma_start(rep_d[k], r_sb[:])

    nc.compile()
    return nc


def host_prep(features, segment_ids, num_segments, weight_matrix, tpc=TPC,
              strict=True):
    """Numpy preprocessing. Returns (nch, in_maps, meta, cnt) or None if the
    geometry (window spans) doesn't fit for this tpc."""
    N, D = features.shape
    G = int(num_segments)
    seg = np.asarray(segment_ids).astype(np.int64)
    feats = np.asarray(features, dtype=np.float32)
    W = np.asarray(weight_matrix, dtype=np.float32)

    chunk = tpc * P
    tiles = tpc + 1
    nodes = tiles * P
    n_pt = (tpc // 2 + 3) // 4

    bnd = np.searchsorted(seg, np.arange(G + 1))
    cnt = np.diff(bnd)
    if cnt.max() > P:
        assert not strict, f"segment with {cnt.max()} nodes > {P}"
        return None
    recip_full = np.where(cnt > 0, 1.0 / np.maximum(cnt, 1), 0.0).astype(np.float32)

    cuts = [0]
    for c in range(1, NCORES):
        gidx = min(int(np.searchsorted(bnd, round(c * N / NCORES))), G)
        cuts.append(int(bnd[gidx]))
    cuts.append(N)
    counts = [cuts[c + 1] - cuts[c] for c in range(NCORES)]
    nch = max(1, math.ceil(max(counts) / chunk))

    in_maps = []
    meta = []
    for c in range(NCORES):
        n0, n1 = cuts[c], cuts[c + 1]
        Nc = n1 - n0
        segl = seg[n0:n1]

        f_pad = np.zeros((P + nch * chunk, D), BF16)
        f_pad[P:P + Nc] = feats[n0:n1].astype(BF16)
        f_in = np.lib.stride_tricks.sliding_window_view(
            f_pad, (nodes, D))[::chunk, 0][:nch]
        f_in = np.ascontiguousarray(
            f_in.reshape(nch, tiles, P, D).transpose(0, 2, 1, 3))

        v = np.arange(Nc)
        chunk_of = v // chunk
        g_lo, g_hi = int(segl[0]), int(segl[-1]) + 1
        own = (bnd[np.arange(g_lo, g_hi) + 1] - 1 - n0) // chunk
        own_of_node = own[segl - g_lo]
        valid = own_of_node == chunk_of

        pw = np.full((nch, tpc // 2), 0, np.int64)
        for k in range(nch):
            for u in range(tpc // 2):
                i = k * chunk + u * 2 * P
                pw[k, u] = segl[min(i, Nc - 1)]

        # quarter-window bases: wq[k, q] = min valid seg in (chunk, quarter)
        tq = tpc // 4
        qn = ((v % chunk) // P) // tq
        wq = np.full((nch, 4), 1 << 40, np.int64)
        np.minimum.at(wq, (chunk_of[valid], qn[valid]), segl[valid])

        relp = np.where(valid, segl - pw[chunk_of, ((v % chunk) // P) // 2],
                        MASK).astype(np.float32)
        relq = np.where(valid, segl - wq[chunk_of, qn], MASK
                        ).astype(np.float32)

        rel32 = np.full((nch, P, tiles), MASK, np.float32)
        brow = np.full((nch, tiles * P), MASK, np.float32)
        pad = np.full(nch * chunk - Nc, MASK, np.float32)
        rp = np.concatenate([relp, pad]).reshape(nch, tpc, P)
        rw = np.concatenate([relq, pad]).reshape(nch, tpc, P)
        rel32[:, :, 1:] = rp.transpose(0, 2, 1)
        brow[:, P:] = rw.reshape(nch, -1)

        for k in range(1, nch):
            lo = k * chunk - P
            if lo >= Nc:
                continue
            hi = min(k * chunk, Nc)
            idx = np.arange(lo, hi)
            bvalid = own_of_node[idx] == k
            if bvalid.any():
                wq[k, 0] = min(wq[k, 0], int(segl[idx][bvalid].min()))
            br = np.where(bvalid, segl[idx] - wq[k, 0], MASK
                          ).astype(np.float32)
            rel32[k, :hi - lo, 0] = br
            brow[k, :hi - lo] = br

        # boundary nodes share quarter 0: re-derive new-node relq of q0
        # against the (possibly lowered) wq[:, 0]
        q0 = valid & (qn == 0)
        relq2 = segl[q0] - wq[chunk_of[q0], 0]
        rw2 = np.full(nch * chunk, MASK, np.float32)
        rw2[:Nc][q0] = relq2
        rw2v = rw2.reshape(nch, tpc, P)
        mask_q0 = np.zeros((nch, tpc, P), bool)
        mask_q0[:, :tq, :] = True
        brow_v = brow[:, P:].reshape(nch, tpc, P)
        brow_v[mask_q0] = rw2v[mask_q0]

        # geometry checks (fall back to smaller tpc on overflow)
        rel_ok = rel32[rel32 > MASK / 2]
        brow_ok = brow[brow > MASK / 2]
        bad = bool(rel_ok.size and (rel_ok.min() < 0
                                    or rel_ok.max() >= PAIRW)) or \
            bool(brow_ok.size and (brow_ok.min() < 0
                                   or brow_ok.max() >= PAIRW))
        if bad:
            assert not strict, "window overflow"
            return None

        relrow = brow.reshape(nch, 1, tiles * P).astype(BF16)

        # merge matrix: slot (pair u, j) -> window row 32q + (seg - wq[q]),
        # pre-scaled by 1/count so ctx = tanh(sum @ W) directly
        m01 = np.zeros((nch, P, n_pt, WINW), np.float32)
        sl = np.arange(P)
        for pt in range(n_pt):
            u = np.minimum(4 * pt + sl // PAIRW, tpc // 2 - 1)
            sseg = pw[:, u] + (sl % PAIRW)[None, :]       # [nch, P]
            okseg = sseg < G
            rec = np.where(okseg, recip_full[np.minimum(sseg, G - 1)], 0.0)
            for q in range(4):
                r = sseg - wq[:, q][:, None]              # [nch, P]
                hit = (r >= 0) & (r < PAIRW) & okseg
                kk, ss = np.nonzero(hit)
                m01[kk, ss, pt, PAIRW * q + r[kk, ss]] = rec[kk, ss]
        m01 = m01.astype(BF16)

        oh01 = (rel32[..., None] ==
                np.arange(PAIRW, dtype=np.float32)).astype(BF16)

        in_maps.append({
            "f_in": f_in,
            "oh01": oh01,
            "relrow": relrow,
            "m01": m01,
            "w_in": W.astype(BF16),
            "piota": np.arange(P, dtype=np.float32)[:, None],
        })
        meta.append({"n0": n0, "n1": n1, "g_lo": g_lo, "g_hi": g_hi,
                     "own": own, "wq": wq, "pw": pw, "tpc": tpc})
    return nch, in_maps, meta, cnt


def assemble(results, meta, G, D, cnt=None):
    rep = np.zeros((G, D), np.float32)
    for c in range(NCORES):
        out = np.asarray(results[c]["rep_out"], dtype=np.float32)
        m = meta[c]
        tpc = m["tpc"]
        n_pt = (tpc // 2 + 3) // 4
        pw = m["pw"]
        nch = pw.shape[0]
        s = np.arange(P)
        u = np.minimum((s // PAIRW)[None, :] + 4 * np.arange(n_pt)[:, None],
                       tpc // 2 - 1)
        tgt = pw[:, u] + (s % PAIRW)[None, None, :]      # [nch, n_pt, P]
        part = out.transpose(0, 2, 1, 3).reshape(nch * n_pt * P, D)
        tgt = tgt.transpose(0, 1, 2).reshape(-1)
        ok = tgt < G
        np.add.at(rep, tgt[ok], part[ok])
    return rep


_LAST_RUN = {}


def kernel(features, segment_ids, num_segments, weight_matrix):
    from concourse.bass_utils import run_bass_kernel_spmd
    _ensure_ntff_hook()

    G = int(num_segments)
    D = features.shape[1]
    prep = host_prep(features, segment_ids, num_segments, weight_matrix,
                     tpc=32, strict=False)
    tpc = 32
    if prep is None:
        tpc = 16
        prep = host_prep(features, segment_ids, num_segments, weight_matrix,
                         tpc=16, strict=True)
    nch, in_maps, meta, cnt = prep
    nc = build_program(nch, tpc)
    trace = bool(int(os.environ.get("BASS_KERNEL_TRACE", "0")))
    kw = {}
    if trace:
        kw["trace"] = True
        kw["tmpdir"] = os.environ.get("BASS_KERNEL_TRACE_DIR") or None
    res = run_bass_kernel_spmd(nc, in_maps, core_ids=list(range(NCORES)), **kw)
    _LAST_RUN["exec_time_ns"] = res.exec_time_ns
    _LAST_RUN["res"] = res
    return assemble(res.results, meta, G, D, cnt)



# revision 36
# speedup vs baseline: 1.2279x; 1.2279x over previous
"""Trainium2 Bass kernel for nn_AttentionModule (segment_reduce).

Computation (per reference):
    wx   = features @ W
    s_g  = segment_sum(wx);  cnt_g = segment counts
    ctx  = tanh(s_g / max(cnt,1))            [G, D]
    score_n = <f_n, ctx[seg_n]>
    rep_g = segment_sum(score_n * f_n)       [G, D]

Key identity: segment_sum(F @ W) = segment_sum(F) @ W, so pass A is a pure
segment-sum.  Device algorithm (SPMD over 8 cores, segment-aligned shard
per core, one uniform compiled program; all data-dependence flows through
input tensors, never instruction addresses):

  per 2048-node chunk (16 new 128-node tiles + 1 boundary re-read tile):
    - one-hot (node x 32-wide pair window) built on DVE by comparing an
      iota const against host-provided window-relative ids
    - segment-sum via PE matmuls into 32-aligned psum slot ranges;
      slot->window merge matmul with an on-device generated 0/1 matrix
    - ctx = tanh((sum @ W) * -recip) (negated; compensates the negated
      transposed one-hot below)
    - transposed one-hot ohT[w, n] = -(relrow[n]==w) via a K=2 broadcast
      matmul (diff = relrow[n] - w), ACT Square, GPSIMD min(d^2-1, 0)
    - CtxG = ohT.T @ (-ctx) (PE), scores = rowsum(F * CtxG) (DVE),
      rep via one-hot matmul + merge, written chunk-major to DRAM
  host: numpy pre/post-processing (index metadata, shard assembly).
"""

import os
import sys
import math
from functools import lru_cache

for _p in ("/opt/trn_rl_repo", "/root/.axon_site/_ro/trn_rl_repo"):
    if os.path.isdir(_p) and _p not in sys.path:
        sys.path.insert(0, _p)

import numpy as np
import ml_dtypes

BF16 = ml_dtypes.bfloat16

P = 128          # partitions
TPC = 16         # new 128-node tiles per chunk
CHUNK = TPC * P  # 2048 new nodes per chunk
TILES = TPC + 1  # + boundary tile (tile 0)
NODES = TILES * P  # nodes touched per chunk (2176)
WINW = 128       # chunk segment-window width
PAIRW = 32       # pair segment-window width
NCORES = 8
MASK = -1000.0


def _ensure_ntff_hook():
    """Register the axon NTFF profile hook if the boot couldn't (antenv stub)."""
    import types
    try:
        import antenv  # noqa
    except ImportError:
        return
    if "antenv.axon_hooks" in sys.modules:
        return
    hooks = types.ModuleType("antenv.axon_hooks")
    holder = [None]
    hooks.set_axon_ntff_profile_hook = lambda h: holder.__setitem__(0, h)
    hooks.get_axon_ntff_profile_hook = lambda: holder[0]
    sys.modules["antenv.axon_hooks"] = hooks
    import antenv
    antenv.axon_hooks = hooks
    try:
        from trn_agent_boot.trn_boot import _ntff_profile_via_ctypes
        so = "/opt/axon/libaxon_pjrt.so"
        if os.path.exists(so):
            hooks.set_axon_ntff_profile_hook(_ntff_profile_via_ctypes(so))
    except Exception:
        pass


@lru_cache(maxsize=4)
def build_program(nch: int, tpc: int = TPC):
    """Build + compile the uniform per-core Bass program.

    Returns (nc, io_names) where io_names lists the dram tensor names.
    """
    import concourse.bass as bass  # noqa
    import concourse.mybir as mybir
    from concourse import bacc, tile

    tiles = tpc + 1
    nodes = tiles * P
    n_pairs = tpc // 2
    n_pt = (n_pairs + 3) // 4          # psum partial tiles (4 pairs each)
    n_g4 = (tiles + 3) // 4            # ctx-gather groups of 4 tiles
    tq = tpc // 4                      # tiles per quarter-window
    f32 = mybir.dt.float32
    bf16 = mybir.dt.bfloat16

    nc = bacc.Bacc("TRN2", target_bir_lowering=False, debug=False,
                   num_devices=NCORES)

    # ---- DRAM I/O ----
    F_d = nc.dram_tensor("f_in", [nch, P, tiles, P], bf16,
                         kind="ExternalInput").ap()
    oh01_d = nc.dram_tensor("oh01", [nch, P, tiles, PAIRW], bf16,
                            kind="ExternalInput").ap()
    relrow_d = nc.dram_tensor("relrow", [nch, 1, nodes], bf16,
                              kind="ExternalInput").ap()
    m01_d = nc.dram_tensor("m01", [nch, P, n_pt, WINW], bf16,
                           kind="ExternalInput").ap()
    piota_d = nc.dram_tensor("piota", [P, 1], mybir.dt.float32,
                             kind="ExternalInput").ap()
    W_d = nc.dram_tensor("w_in", [P, P], bf16, kind="ExternalInput").ap()
    rep_d = nc.dram_tensor("rep_out", [nch, P, n_pt, P], bf16,
                           kind="ExternalOutput").ap()

    AluOp = mybir.AluOpType
    Act = mybir.ActivationFunctionType

    with tile.TileContext(nc) as tc:
        with tc.tile_pool(name="const", bufs=1) as cpool, \
             tc.tile_pool(name="fpool", bufs=4) as fpool, \
             tc.tile_pool(name="small", bufs=4) as spool, \
             tc.tile_pool(name="big", bufs=3) as bpool, \
             tc.tile_pool(name="ps_ctx", bufs=2, space="PSUM") as pss, \
             tc.tile_pool(name="ps_acc", bufs=2, space="PSUM") as psa, \
             tc.tile_pool(name="ps_big", bufs=2, space="PSUM") as psb:

            piota_t = cpool.tile([P, 1], mybir.dt.float32)
            w_t = cpool.tile([P, P], bf16)
            nc.sync.dma_start(piota_t[:], piota_d[:])
            nc.sync.dma_start(w_t[:], W_d[:])

            for k in range(nch):
                # ---- loads ----
                f_t = fpool.tile([P, tiles, P], bf16, tag="f")
                nc.sync.dma_start(f_t[:], F_d[k])

                m01_t = spool.tile([P, n_pt, WINW], bf16, tag="m01")
                nc.sync.dma_start(m01_t[:], m01_d[k])

                # ---- one-hot [node, pair-window] (host-built) ----
                oh_t = spool.tile([P, tiles, PAIRW], bf16, tag="oh")
                nc.sync.dma_start(oh_t[:], oh01_d[k])


                def tile_order():
                    # first MM into each psum partial tile must be a fresh
                    # (start=True) write; boundary tile 0 accumulates into
                    # tile 1's slot range so it must come after tile 1.
                    order = [1, 0] + list(range(2, tiles))
                    return order

                def slot_base(t):
                    u = (t - 1) // 2 if t >= 1 else 0
                    return (u % 4) * PAIRW, u // 4

                # ---- step1: segment sums into slots ----
                order = tile_order()
                # region (pair) u: writers in order; first gets start=True,
                # last gets stop=True.  boundary tile 0 shares region 0.
                region_of = {t: ((t - 1) // 2 if t >= 1 else 0) for t in order}
                first_of_region = {}
                last_of_region = {}
                for t in order:
                    u = region_of[t]
                    first_of_region.setdefault(u, t)
                    last_of_region[u] = t
                ps_s = psa.tile([P, n_pt, P], f32, tag="ps_s",
                                name=f"ps_s_{k}")
                for t in order:
                    base, pt = slot_base(t)
                    u = region_of[t]
                    nc.tensor.matmul(
                        ps_s[base:base + PAIRW, pt, :],
                        oh_t[:, t, :], f_t[:, t, :],
                        start=(first_of_region[u] == t),
                        stop=(last_of_region[u] == t),
                        tile_position=(0, base))

                # ---- merge to sumT [D, w] ----
                ps_sumT = pss.tile([P, WINW], f32, tag="ctxps")
                s_sb = spool.tile([P, n_pt, P], bf16, tag="s_sb")
                nc.scalar.copy(s_sb[:], ps_s[:])
                for pt in range(n_pt):
                    nc.tensor.matmul(ps_sumT[:], s_sb[:, pt, :], m01_t[:, pt, :],
                                     start=(pt == 0), stop=(pt == n_pt - 1))

                # ---- ctx: pre[q] = (recip*sum)[32q:32(q+1)] @ W, quarter-
                # aligned at partitions 0:32 so the gather contracts over
                # K=32 (recip is folded into m01 on the host) ----
                sumT_sb = spool.tile([P, WINW], bf16, tag="sumT_sb")
                nc.scalar.copy(sumT_sb[:], ps_sumT[:])
                ps_preq = pss.tile([PAIRW, 4, P], f32, tag="ctxps")
                for q in range(4):
                    nc.tensor.matmul(ps_preq[:, q, :],
                                     sumT_sb[:, q * PAIRW:(q + 1) * PAIRW],
                                     w_t[:], start=True, stop=True)
                ctxq_t = spool.tile([PAIRW, 4, P], bf16, tag="ctxq")
                nc.scalar.activation(ctxq_t[:], ps_preq[:], Act.Tanh)

                # ---- ohT[j, n] = (relrow[n] == j), quarter-local ----
                relb_t = bpool.tile([PAIRW, nodes], bf16, tag="relb")
                nc.sync.dma_start(relb_t[:],
                                  relrow_d[k].broadcast_to([PAIRW, nodes]))
                ohT_t = bpool.tile([PAIRW, nodes], bf16, tag="ohT")
                nc.vector.tensor_scalar(out=ohT_t[:], in0=relb_t[:],
                                        scalar1=piota_t[0:PAIRW, :],
                                        scalar2=None,
                                        op0=AluOp.is_equal)

                # ---- 3a: CtxG groups + scores ----
                def quarter_of(t):
                    return 0 if t == 0 else min((t - 1) // tq, 3)

                prod_sb = bpool.tile([P, tiles, P], bf16, tag="prod")
                n_act = 7          # gather groups routed psum->sbuf via ACT
                for g in range(n_g4):
                    t0g = g * 4
                    ng = min(4, tiles - t0g)
                    ps_cg = psb.tile([P, 512], f32, tag="bigps")
                    for i in range(ng):
                        t = t0g + i
                        nc.tensor.matmul(
                            ps_cg[:, i * P:(i + 1) * P],
                            ohT_t[:, t * P:(t + 1) * P],
                            ctxq_t[:, quarter_of(t), :],
                            start=(i == 0), stop=(i == ng - 1))
                    if g < n_act:
                        # ACT copies psum->sbuf bf16 so the DVE multiply
                        # runs in 2x mode (all-2-byte operands)
                        cg_sb = spool.tile([P, 512], bf16, tag="cg_sb")
                        nc.scalar.copy(cg_sb[:, :ng * P], ps_cg[:, :ng * P])
                        nc.vector.tensor_tensor(
                            out=prod_sb[:, t0g:t0g + ng, :],
                            in0=f_t[:, t0g:t0g + ng, :],
                            in1=cg_sb[:, :ng * P].rearrange(
                                "p (t d) -> p t d", d=P),
                            op=AluOp.mult)
                    else:
                        nc.vector.tensor_tensor(
                            out=prod_sb[:, t0g:t0g + ng, :],
                            in0=f_t[:, t0g:t0g + ng, :],
                            in1=ps_cg[:, :ng * P].rearrange(
                                "p (t d) -> p t d", d=P),
                            op=AluOp.mult)
                fold_sb = spool.tile([P, tiles, P // 2], bf16, tag="fold")
                nc.vector.tensor_tensor(
                    out=fold_sb[:], in0=prod_sb[:, :, :P // 2],
                    in1=prod_sb[:, :, P // 2:], op=AluOp.add)
                fold2_sb = spool.tile([P, tiles, P // 4], bf16, tag="fold2")
                nc.vector.tensor_tensor(
                    out=fold2_sb[:], in0=fold_sb[:, :, :P // 4],
                    in1=fold_sb[:, :, P // 4:], op=AluOp.add)
                scores_b = spool.tile([P, tiles], bf16, tag="scores_b")
                with nc.allow_low_precision(
                        reason="scores reduce: bf16 out is within budget"):
                    nc.vector.tensor_reduce(out=scores_b[:], in_=fold2_sb[:],
                                            axis=mybir.AxisListType.X,
                                            op=AluOp.add)

                # ---- 3b: rep sums ----
                ohsc_t = spool.tile([P, tiles, PAIRW], bf16, tag="ohsc")
                nc.vector.tensor_tensor(
                    out=ohsc_t[:], in0=oh_t[:],
                    in1=scores_b[:].unsqueeze(2).broadcast_to(
                        [P, tiles, PAIRW]),
                    op=AluOp.mult)
                ps_r = psa.tile([P, n_pt, P], f32, tag="ps_r",
                                name=f"ps_r_{k}")
                for t in order:
                    base, pt = slot_base(t)
                    u = region_of[t]
                    nc.tensor.matmul(
                        ps_r[base:base + PAIRW, pt, :],
                        ohsc_t[:, t, :], f_t[:, t, :],
                        start=(first_of_region[u] == t),
                        stop=(last_of_region[u] == t),
                        tile_position=(0, base))
                r_sb = spool.tile([P, n_pt, P], bf16, tag="r_sb")
                nc.scalar.copy(r_sb[:], ps_r[:])
                nc.sync.dma_start(rep_d[k], r_sb[:])

    nc.compile()
    return nc


def host_prep(features, segment_ids, num_segments, weight_matrix, tpc=TPC,
              strict=True):
    """Numpy preprocessing. Returns (nch, in_maps, meta, cnt) or None if the
    geometry (window spans) doesn't fit for this tpc."""
    N, D = features.shape
    G = int(num_segments)
    seg = np.asarray(segment_ids).astype(np.int64)
    feats = np.asarray(features, dtype=np.float32)
    W = np.asarray(weight_matrix, dtype=np.float32)

    chunk = tpc * P
    tiles = tpc + 1
    nodes = tiles * P
    n_pt = (tpc // 2 + 3) // 4

    bnd = np.searchsorted(seg, np.arange(G + 1))
    cnt = np.diff(bnd)
    if cnt.max() > P:
        assert not strict, f"segment with {cnt.max()} nodes > {P}"
        return None
    recip_full = np.where(cnt > 0, 1.0 / np.maximum(cnt, 1), 0.0).astype(np.float32)

    cuts = [0]
    for c in range(1, NCORES):
        gidx = min(int(np.searchsorted(bnd, round(c * N / NCORES))), G)
        cuts.append(int(bnd[gidx]))
    cuts.append(N)
    counts = [cuts[c + 1] - cuts[c] for c in range(NCORES)]
    nch = max(1, math.ceil(max(counts) / chunk))

    in_maps = []
    meta = []
    for c in range(NCORES):
        n0, n1 = cuts[c], cuts[c + 1]
        Nc = n1 - n0
        segl = seg[n0:n1]

        f_pad = np.zeros((P + nch * chunk, D), BF16)
        f_pad[P:P + Nc] = feats[n0:n1].astype(BF16)
        f_in = np.lib.stride_tricks.sliding_window_view(
            f_pad, (nodes, D))[::chunk, 0][:nch]
        f_in = np.ascontiguousarray(
            f_in.reshape(nch, tiles, P, D).transpose(0, 2, 1, 3))

        v = np.arange(Nc)
        chunk_of = v // chunk
        g_lo, g_hi = int(segl[0]), int(segl[-1]) + 1
        own = (bnd[np.arange(g_lo, g_hi) + 1] - 1 - n0) // chunk
        own_of_node = own[segl - g_lo]
        valid = own_of_node == chunk_of

        pw = np.full((nch, tpc // 2), 0, np.int64)
        for k in range(nch):
            for u in range(tpc // 2):
                i = k * chunk + u * 2 * P
                pw[k, u] = segl[min(i, Nc - 1)]

        # quarter-window bases: wq[k, q] = min valid seg in (chunk, quarter)
        tq = tpc // 4
        qn = ((v % chunk) // P) // tq
        wq = np.full((nch, 4), 1 << 40, np.int64)
        np.minimum.at(wq, (chunk_of[valid], qn[valid]), segl[valid])

        relp = np.where(valid, segl - pw[chunk_of, ((v % chunk) // P) // 2],
                        MASK).astype(np.float32)
        relq = np.where(valid, segl - wq[chunk_of, qn], MASK
                        ).astype(np.float32)

        rel32 = np.full((nch, P, tiles), MASK, np.float32)
        brow = np.full((nch, tiles * P), MASK, np.float32)
        pad = np.full(nch * chunk - Nc, MASK, np.float32)
        rp = np.concatenate([relp, pad]).reshape(nch, tpc, P)
        rw = np.concatenate([relq, pad]).reshape(nch, tpc, P)
        rel32[:, :, 1:] = rp.transpose(0, 2, 1)
        brow[:, P:] = rw.reshape(nch, -1)

        for k in range(1, nch):
            lo = k * chunk - P
            if lo >= Nc:
                continue
            hi = min(k * chunk, Nc)
            idx = np.arange(lo, hi)
            bvalid = own_of_node[idx] == k
            if bvalid.any():
                wq[k, 0] = min(wq[k, 0], int(segl[idx][bvalid].min()))
            br = np.where(bvalid, segl[idx] - wq[k, 0], MASK
                          ).astype(np.float32)
            rel32[k, :hi - lo, 0] = br
            brow[k, :hi - lo] = br

        # boundary nodes share quarter 0: re-derive new-node relq of q0
        # against the (possibly lowered) wq[:, 0]
        q0 = valid & (qn == 0)
        relq2 = segl[q0] - wq[chunk_of[q0], 0]
        rw2 = np.full(nch * chunk, MASK, np.float32)
        rw2[:Nc][q0] = relq2
        rw2v = rw2.reshape(nch, tpc, P)
        mask_q0 = np.zeros((nch, tpc, P), bool)
        mask_q0[:, :tq, :] = True
        brow_v = brow[:, P:].reshape(nch, tpc, P)
        brow_v[mask_q0] = rw2v[mask_q0]

        # geometry checks (fall back to smaller tpc on overflow)
        rel_ok = rel32[rel32 > MASK / 2]
        brow_ok = brow[brow > MASK / 2]
        bad = bool(rel_ok.size and (rel_ok.min() < 0
                                    or rel_ok.max() >= PAIRW)) or \
            bool(brow_ok.size and (brow_ok.min() < 0
                                   or brow_ok.max() >= PAIRW))
        if bad:
            assert not strict, "window overflow"
            return None

        relrow = brow.reshape(nch, 1, tiles * P).astype(BF16)

        # merge matrix: slot (pair u, j) -> window row 32q + (seg - wq[q]),
        # pre-scaled by 1/count so ctx = tanh(sum @ W) directly
        m01 = np.zeros((nch, P, n_pt, WINW), np.float32)
        sl = np.arange(P)
        for pt in range(n_pt):
            u = np.minimum(4 * pt + sl // PAIRW, tpc // 2 - 1)
            sseg = pw[:, u] + (sl % PAIRW)[None, :]       # [nch, P]
            okseg = sseg < G
            rec = np.where(okseg, recip_full[np.minimum(sseg, G - 1)], 0.0)
            for q in range(4):
                r = sseg - wq[:, q][:, None]              # [nch, P]
                hit = (r >= 0) & (r < PAIRW) & okseg
                kk, ss = np.nonzero(hit)
                m01[kk, ss, pt, PAIRW * q + r[kk, ss]] = rec[kk, ss]
        m01 = m01.astype(BF16)

        oh01 = (rel32[..., None] ==
                np.arange(PAIRW, dtype=np.float32)).astype(BF16)

        in_maps.append({
            "f_in": f_in,
            "oh01": oh01,
            "relrow": relrow,
            "m01": m01,
            "w_in": W.astype(BF16),
            "piota": np.arange(P, dtype=np.float32)[:, None],
        })
        meta.append({"n0": n0, "n1": n1, "g_lo": g_lo, "g_hi": g_hi,
                     "own": own, "wq": wq, "pw": pw, "tpc": tpc})
    return nch, in_maps, meta, cnt


def assemble(results, meta, G, D, cnt=None):
    rep = np.zeros((G, D), np.float32)
    for c in range(NCORES):
        out = np.asarray(results[c]["rep_out"], dtype=np.float32)
        m = meta[c]
        tpc = m["tpc"]
        n_pt = (tpc // 2 + 3) // 4
        pw = m["pw"]
        nch = pw.shape[0]
        s = np.arange(P)
        u = np.minimum((s // PAIRW)[None, :] + 4 * np.arange(n_pt)[:, None],
                       tpc // 2 - 1)
        tgt = pw[:, u] + (s % PAIRW)[None, None, :]      # [nch, n_pt, P]
        part = out.transpose(0, 2, 1, 3).reshape(nch * n_pt * P, D)
        tgt = tgt.transpose(0, 1, 2).reshape(-1)
        ok = tgt < G
        np.add.at(rep, tgt[ok], part[ok])
    return rep


_LAST_RUN = {}


def kernel(features, segment_ids, num_segments, weight_matrix):
    from concourse.bass_utils import run_bass_kernel_spmd
    _ensure_ntff_hook()

    G = int(num_segments)
    D = features.shape[1]
    prep = host_prep(features, segment_ids, num_segments, weight_matrix,
                     tpc=32, strict=False)
    tpc = 32
    if prep is None:
        tpc = 16
        prep = host_prep(features, segment_ids, num_segments, weight_matrix,
                         tpc=16, strict=True)
    nch, in_maps, meta, cnt = prep
    nc = build_program(nch, tpc)
    trace = bool(int(os.environ.get("BASS_KERNEL_TRACE", "0")))
    kw = {}
    if trace:
        kw["trace"] = True
        kw["tmpdir"] = os.environ.get("BASS_KERNEL_TRACE_DIR") or None
    res = run_bass_kernel_spmd(nc, in_maps, core_ids=list(range(NCORES)), **kw)
    _LAST_RUN["exec_time_ns"] = res.exec_time_ns
    _LAST_RUN["res"] = res
    return assemble(res.results, meta, G, D, cnt)



# revision 37
# speedup vs baseline: 1.2319x; 1.0033x over previous
"""Trainium2 Bass kernel for nn_AttentionModule (segment_reduce).

Computation (per reference):
    wx   = features @ W
    s_g  = segment_sum(wx);  cnt_g = segment counts
    ctx  = tanh(s_g / max(cnt,1))            [G, D]
    score_n = <f_n, ctx[seg_n]>
    rep_g = segment_sum(score_n * f_n)       [G, D]

Key identity: segment_sum(F @ W) = segment_sum(F) @ W, so pass A is a pure
segment-sum.  Device algorithm (SPMD over 8 cores, segment-aligned shard
per core, one uniform compiled program; all data-dependence flows through
input tensors, never instruction addresses):

  per 2048-node chunk (16 new 128-node tiles + 1 boundary re-read tile):
    - one-hot (node x 32-wide pair window) built on DVE by comparing an
      iota const against host-provided window-relative ids
    - segment-sum via PE matmuls into 32-aligned psum slot ranges;
      slot->window merge matmul with an on-device generated 0/1 matrix
    - ctx = tanh((sum @ W) * -recip) (negated; compensates the negated
      transposed one-hot below)
    - transposed one-hot ohT[w, n] = -(relrow[n]==w) via a K=2 broadcast
      matmul (diff = relrow[n] - w), ACT Square, GPSIMD min(d^2-1, 0)
    - CtxG = ohT.T @ (-ctx) (PE), scores = rowsum(F * CtxG) (DVE),
      rep via one-hot matmul + merge, written chunk-major to DRAM
  host: numpy pre/post-processing (index metadata, shard assembly).
"""

import os
import sys
import math
from functools import lru_cache

for _p in ("/opt/trn_rl_repo", "/root/.axon_site/_ro/trn_rl_repo"):
    if os.path.isdir(_p) and _p not in sys.path:
        sys.path.insert(0, _p)

import numpy as np
import ml_dtypes

BF16 = ml_dtypes.bfloat16

P = 128          # partitions
TPC = 16         # new 128-node tiles per chunk
CHUNK = TPC * P  # 2048 new nodes per chunk
TILES = TPC + 1  # + boundary tile (tile 0)
NODES = TILES * P  # nodes touched per chunk (2176)
WINW = 128       # chunk segment-window width
PAIRW = 32       # pair segment-window width
NCORES = 8
MASK = -1000.0


def _ensure_ntff_hook():
    """Register the axon NTFF profile hook if the boot couldn't (antenv stub)."""
    import types
    try:
        import antenv  # noqa
    except ImportError:
        return
    if "antenv.axon_hooks" in sys.modules:
        return
    hooks = types.ModuleType("antenv.axon_hooks")
    holder = [None]
    hooks.set_axon_ntff_profile_hook = lambda h: holder.__setitem__(0, h)
    hooks.get_axon_ntff_profile_hook = lambda: holder[0]
    sys.modules["antenv.axon_hooks"] = hooks
    import antenv
    antenv.axon_hooks = hooks
    try:
        from trn_agent_boot.trn_boot import _ntff_profile_via_ctypes
        so = "/opt/axon/libaxon_pjrt.so"
        if os.path.exists(so):
            hooks.set_axon_ntff_profile_hook(_ntff_profile_via_ctypes(so))
    except Exception:
        pass


@lru_cache(maxsize=4)
def build_program(nch: int, tpc: int = TPC):
    """Build + compile the uniform per-core Bass program.

    Returns (nc, io_names) where io_names lists the dram tensor names.
    """
    import concourse.bass as bass  # noqa
    import concourse.mybir as mybir
    from concourse import bacc, tile

    tiles = tpc + 1
    nodes = tiles * P
    n_pairs = tpc // 2
    n_pt = (n_pairs + 3) // 4          # psum partial tiles (4 pairs each)
    n_g4 = (tiles + 3) // 4            # ctx-gather groups of 4 tiles
    tq = tpc // 4                      # tiles per quarter-window
    f32 = mybir.dt.float32
    bf16 = mybir.dt.bfloat16

    nc = bacc.Bacc("TRN2", target_bir_lowering=False, debug=False,
                   num_devices=NCORES)

    # ---- DRAM I/O ----
    F_d = nc.dram_tensor("f_in", [nch, P, tiles, P], bf16,
                         kind="ExternalInput").ap()
    oh01_d = nc.dram_tensor("oh01", [nch, P, tiles, PAIRW], bf16,
                            kind="ExternalInput").ap()
    relrow_d = nc.dram_tensor("relrow", [nch, 1, nodes], bf16,
                              kind="ExternalInput").ap()
    m01_d = nc.dram_tensor("m01", [nch, P, n_pt, WINW], bf16,
                           kind="ExternalInput").ap()
    piota_d = nc.dram_tensor("piota", [P, 1], mybir.dt.float32,
                             kind="ExternalInput").ap()
    W_d = nc.dram_tensor("w_in", [P, P], bf16, kind="ExternalInput").ap()
    rep_d = nc.dram_tensor("rep_out", [nch, P, n_pt, P], bf16,
                           kind="ExternalOutput").ap()

    AluOp = mybir.AluOpType
    Act = mybir.ActivationFunctionType

    with tile.TileContext(nc) as tc:
        with tc.tile_pool(name="const", bufs=1) as cpool, \
             tc.tile_pool(name="fpool", bufs=4) as fpool, \
             tc.tile_pool(name="small", bufs=4) as spool, \
             tc.tile_pool(name="big", bufs=4) as bpool, \
             tc.tile_pool(name="ps_ctx", bufs=2, space="PSUM") as pss, \
             tc.tile_pool(name="ps_acc", bufs=2, space="PSUM") as psa, \
             tc.tile_pool(name="ps_big", bufs=2, space="PSUM") as psb:

            piota_t = cpool.tile([P, 1], mybir.dt.float32)
            w_t = cpool.tile([P, P], bf16)
            nc.sync.dma_start(piota_t[:], piota_d[:])
            nc.sync.dma_start(w_t[:], W_d[:])

            for k in range(nch):
                # ---- loads ----
                f_t = fpool.tile([P, tiles, P], bf16, tag="f")
                nc.sync.dma_start(f_t[:], F_d[k])

                m01_t = spool.tile([P, n_pt, WINW], bf16, tag="m01")
                nc.sync.dma_start(m01_t[:], m01_d[k])

                # ---- one-hot [node, pair-window] (host-built) ----
                oh_t = spool.tile([P, tiles, PAIRW], bf16, tag="oh")
                nc.sync.dma_start(oh_t[:], oh01_d[k])


                def tile_order():
                    # first MM into each psum partial tile must be a fresh
                    # (start=True) write; boundary tile 0 accumulates into
                    # tile 1's slot range so it must come after tile 1.
                    order = [1, 0] + list(range(2, tiles))
                    return order

                def slot_base(t):
                    u = (t - 1) // 2 if t >= 1 else 0
                    return (u % 4) * PAIRW, u // 4

                # ---- step1: segment sums into slots ----
                order = tile_order()
                # region (pair) u: writers in order; first gets start=True,
                # last gets stop=True.  boundary tile 0 shares region 0.
                region_of = {t: ((t - 1) // 2 if t >= 1 else 0) for t in order}
                first_of_region = {}
                last_of_region = {}
                for t in order:
                    u = region_of[t]
                    first_of_region.setdefault(u, t)
                    last_of_region[u] = t
                ps_s = psa.tile([P, n_pt, P], f32, tag="ps_s",
                                name=f"ps_s_{k}")
                for t in order:
                    base, pt = slot_base(t)
                    u = region_of[t]
                    nc.tensor.matmul(
                        ps_s[base:base + PAIRW, pt, :],
                        oh_t[:, t, :], f_t[:, t, :],
                        start=(first_of_region[u] == t),
                        stop=(last_of_region[u] == t),
                        tile_position=(0, base))

                # ---- merge to sumT [D, w] ----
                ps_sumT = pss.tile([P, WINW], f32, tag="ctxps")
                s_sb = spool.tile([P, n_pt, P], bf16, tag="s_sb")
                nc.scalar.copy(s_sb[:], ps_s[:])
                for pt in range(n_pt):
                    nc.tensor.matmul(ps_sumT[:], s_sb[:, pt, :], m01_t[:, pt, :],
                                     start=(pt == 0), stop=(pt == n_pt - 1))

                # ---- ctx: pre[q] = (recip*sum)[32q:32(q+1)] @ W, quarter-
                # aligned at partitions 0:32 so the gather contracts over
                # K=32 (recip is folded into m01 on the host) ----
                sumT_sb = spool.tile([P, WINW], bf16, tag="sumT_sb")
                nc.scalar.copy(sumT_sb[:], ps_sumT[:])
                ps_preq = pss.tile([PAIRW, 4, P], f32, tag="ctxps")
                for q in range(4):
                    nc.tensor.matmul(ps_preq[:, q, :],
                                     sumT_sb[:, q * PAIRW:(q + 1) * PAIRW],
                                     w_t[:], start=True, stop=True)
                ctxq_t = spool.tile([PAIRW, 4, P], bf16, tag="ctxq")
                nc.scalar.activation(ctxq_t[:], ps_preq[:], Act.Tanh)

                # ---- ohT[j, n] = (relrow[n] == j), quarter-local ----
                relb_t = bpool.tile([PAIRW, nodes], bf16, tag="relb")
                nc.sync.dma_start(relb_t[:],
                                  relrow_d[k].broadcast_to([PAIRW, nodes]))
                ohT_t = bpool.tile([PAIRW, nodes], bf16, tag="ohT")
                nc.vector.tensor_scalar(out=ohT_t[:], in0=relb_t[:],
                                        scalar1=piota_t[0:PAIRW, :],
                                        scalar2=None,
                                        op0=AluOp.is_equal)

                # ---- 3a: CtxG groups + scores ----
                def quarter_of(t):
                    return 0 if t == 0 else min((t - 1) // tq, 3)

                prod_sb = bpool.tile([P, tiles, P], bf16, tag="prod")
                n_act = 7          # gather groups routed psum->sbuf via ACT
                for g in range(n_g4):
                    t0g = g * 4
                    ng = min(4, tiles - t0g)
                    ps_cg = psb.tile([P, 512], f32, tag="bigps")
                    for i in range(ng):
                        t = t0g + i
                        nc.tensor.matmul(
                            ps_cg[:, i * P:(i + 1) * P],
                            ohT_t[:, t * P:(t + 1) * P],
                            ctxq_t[:, quarter_of(t), :],
                            start=(i == 0), stop=(i == ng - 1))
                    if g < n_act:
                        # ACT copies psum->sbuf bf16 so the DVE multiply
                        # runs in 2x mode (all-2-byte operands)
                        cg_sb = spool.tile([P, 512], bf16, tag="cg_sb")
                        nc.scalar.copy(cg_sb[:, :ng * P], ps_cg[:, :ng * P])
                        nc.vector.tensor_tensor(
                            out=prod_sb[:, t0g:t0g + ng, :],
                            in0=f_t[:, t0g:t0g + ng, :],
                            in1=cg_sb[:, :ng * P].rearrange(
                                "p (t d) -> p t d", d=P),
                            op=AluOp.mult)
                    else:
                        nc.vector.tensor_tensor(
                            out=prod_sb[:, t0g:t0g + ng, :],
                            in0=f_t[:, t0g:t0g + ng, :],
                            in1=ps_cg[:, :ng * P].rearrange(
                                "p (t d) -> p t d", d=P),
                            op=AluOp.mult)
                fold_sb = spool.tile([P, tiles, P // 2], bf16, tag="fold")
                nc.vector.tensor_tensor(
                    out=fold_sb[:], in0=prod_sb[:, :, :P // 2],
                    in1=prod_sb[:, :, P // 2:], op=AluOp.add)
                fold2_sb = spool.tile([P, tiles, P // 4], bf16, tag="fold2")
                nc.vector.tensor_tensor(
                    out=fold2_sb[:], in0=fold_sb[:, :, :P // 4],
                    in1=fold_sb[:, :, P // 4:], op=AluOp.add)
                scores_b = spool.tile([P, tiles], bf16, tag="scores_b")
                with nc.allow_low_precision(
                        reason="scores reduce: bf16 out is within budget"):
                    nc.vector.tensor_reduce(out=scores_b[:], in_=fold2_sb[:],
                                            axis=mybir.AxisListType.X,
                                            op=AluOp.add)

                # ---- 3b: rep sums ----
                ohsc_t = spool.tile([P, tiles, PAIRW], bf16, tag="ohsc")
                nc.vector.tensor_tensor(
                    out=ohsc_t[:], in0=oh_t[:],
                    in1=scores_b[:].unsqueeze(2).broadcast_to(
                        [P, tiles, PAIRW]),
                    op=AluOp.mult)
                ps_r = psa.tile([P, n_pt, P], f32, tag="ps_r",
                                name=f"ps_r_{k}")
                for t in order:
                    base, pt = slot_base(t)
                    u = region_of[t]
                    nc.tensor.matmul(
                        ps_r[base:base + PAIRW, pt, :],
                        ohsc_t[:, t, :], f_t[:, t, :],
                        start=(first_of_region[u] == t),
                        stop=(last_of_region[u] == t),
                        tile_position=(0, base))
                r_sb = spool.tile([P, n_pt, P], bf16, tag="r_sb")
                nc.scalar.copy(r_sb[:], ps_r[:])
                nc.sync.dma_start(rep_d[k], r_sb[:])

    nc.compile()
    return nc


def host_prep(features, segment_ids, num_segments, weight_matrix, tpc=TPC,
              strict=True):
    """Numpy preprocessing. Returns (nch, in_maps, meta, cnt) or None if the
    geometry (window spans) doesn't fit for this tpc."""
    N, D = features.shape
    G = int(num_segments)
    seg = np.asarray(segment_ids).astype(np.int64)
    feats = np.asarray(features, dtype=np.float32)
    W = np.asarray(weight_matrix, dtype=np.float32)

    chunk = tpc * P
    tiles = tpc + 1
    nodes = tiles * P
    n_pt = (tpc // 2 + 3) // 4

    bnd = np.searchsorted(seg, np.arange(G + 1))
    cnt = np.diff(bnd)
    if cnt.max() > P:
        assert not strict, f"segment with {cnt.max()} nodes > {P}"
        return None
    recip_full = np.where(cnt > 0, 1.0 / np.maximum(cnt, 1), 0.0).astype(np.float32)

    cuts = [0]
    for c in range(1, NCORES):
        gidx = min(int(np.searchsorted(bnd, round(c * N / NCORES))), G)
        cuts.append(int(bnd[gidx]))
    cuts.append(N)
    counts = [cuts[c + 1] - cuts[c] for c in range(NCORES)]
    nch = max(1, math.ceil(max(counts) / chunk))

    in_maps = []
    meta = []
    for c in range(NCORES):
        n0, n1 = cuts[c], cuts[c + 1]
        Nc = n1 - n0
        segl = seg[n0:n1]

        f_pad = np.zeros((P + nch * chunk, D), BF16)
        f_pad[P:P + Nc] = feats[n0:n1].astype(BF16)
        f_in = np.lib.stride_tricks.sliding_window_view(
            f_pad, (nodes, D))[::chunk, 0][:nch]
        f_in = np.ascontiguousarray(
            f_in.reshape(nch, tiles, P, D).transpose(0, 2, 1, 3))

        v = np.arange(Nc)
        chunk_of = v // chunk
        g_lo, g_hi = int(segl[0]), int(segl[-1]) + 1
        own = (bnd[np.arange(g_lo, g_hi) + 1] - 1 - n0) // chunk
        own_of_node = own[segl - g_lo]
        valid = own_of_node == chunk_of

        pw = np.full((nch, tpc // 2), 0, np.int64)
        for k in range(nch):
            for u in range(tpc // 2):
                i = k * chunk + u * 2 * P
                pw[k, u] = segl[min(i, Nc - 1)]

        # quarter-window bases: wq[k, q] = min valid seg in (chunk, quarter)
        tq = tpc // 4
        qn = ((v % chunk) // P) // tq
        wq = np.full((nch, 4), 1 << 40, np.int64)
        np.minimum.at(wq, (chunk_of[valid], qn[valid]), segl[valid])

        relp = np.where(valid, segl - pw[chunk_of, ((v % chunk) // P) // 2],
                        MASK).astype(np.float32)
        relq = np.where(valid, segl - wq[chunk_of, qn], MASK
                        ).astype(np.float32)

        rel32 = np.full((nch, P, tiles), MASK, np.float32)
        brow = np.full((nch, tiles * P), MASK, np.float32)
        pad = np.full(nch * chunk - Nc, MASK, np.float32)
        rp = np.concatenate([relp, pad]).reshape(nch, tpc, P)
        rw = np.concatenate([relq, pad]).reshape(nch, tpc, P)
        rel32[:, :, 1:] = rp.transpose(0, 2, 1)
        brow[:, P:] = rw.reshape(nch, -1)

        for k in range(1, nch):
            lo = k * chunk - P
            if lo >= Nc:
                continue
            hi = min(k * chunk, Nc)
            idx = np.arange(lo, hi)
            bvalid = own_of_node[idx] == k
            if bvalid.any():
                wq[k, 0] = min(wq[k, 0], int(segl[idx][bvalid].min()))
            br = np.where(bvalid, segl[idx] - wq[k, 0], MASK
                          ).astype(np.float32)
            rel32[k, :hi - lo, 0] = br
            brow[k, :hi - lo] = br

        # boundary nodes share quarter 0: re-derive new-node relq of q0
        # against the (possibly lowered) wq[:, 0]
        q0 = valid & (qn == 0)
        relq2 = segl[q0] - wq[chunk_of[q0], 0]
        rw2 = np.full(nch * chunk, MASK, np.float32)
        rw2[:Nc][q0] = relq2
        rw2v = rw2.reshape(nch, tpc, P)
        mask_q0 = np.zeros((nch, tpc, P), bool)
        mask_q0[:, :tq, :] = True
        brow_v = brow[:, P:].reshape(nch, tpc, P)
        brow_v[mask_q0] = rw2v[mask_q0]

        # geometry checks (fall back to smaller tpc on overflow)
        rel_ok = rel32[rel32 > MASK / 2]
        brow_ok = brow[brow > MASK / 2]
        bad = bool(rel_ok.size and (rel_ok.min() < 0
                                    or rel_ok.max() >= PAIRW)) or \
            bool(brow_ok.size and (brow_ok.min() < 0
                                   or brow_ok.max() >= PAIRW))
        if bad:
            assert not strict, "window overflow"
            return None

        relrow = brow.reshape(nch, 1, tiles * P).astype(BF16)

        # merge matrix: slot (pair u, j) -> window row 32q + (seg - wq[q]),
        # pre-scaled by 1/count so ctx = tanh(sum @ W) directly
        m01 = np.zeros((nch, P, n_pt, WINW), np.float32)
        sl = np.arange(P)
        for pt in range(n_pt):
            u = np.minimum(4 * pt + sl // PAIRW, tpc // 2 - 1)
            sseg = pw[:, u] + (sl % PAIRW)[None, :]       # [nch, P]
            okseg = sseg < G
            rec = np.where(okseg, recip_full[np.minimum(sseg, G - 1)], 0.0)
            for q in range(4):
                r = sseg - wq[:, q][:, None]              # [nch, P]
                hit = (r >= 0) & (r < PAIRW) & okseg
                kk, ss = np.nonzero(hit)
                m01[kk, ss, pt, PAIRW * q + r[kk, ss]] = rec[kk, ss]
        m01 = m01.astype(BF16)

        oh01 = (rel32[..., None] ==
                np.arange(PAIRW, dtype=np.float32)).astype(BF16)

        in_maps.append({
            "f_in": f_in,
            "oh01": oh01,
            "relrow": relrow,
            "m01": m01,
            "w_in": W.astype(BF16),
            "piota": np.arange(P, dtype=np.float32)[:, None],
        })
        meta.append({"n0": n0, "n1": n1, "g_lo": g_lo, "g_hi": g_hi,
                     "own": own, "wq": wq, "pw": pw, "tpc": tpc})
    return nch, in_maps, meta, cnt


def assemble(results, meta, G, D, cnt=None):
    rep = np.zeros((G, D), np.float32)
    for c in range(NCORES):
        out = np.asarray(results[c]["rep_out"], dtype=np.float32)
        m = meta[c]
        tpc = m["tpc"]
        n_pt = (tpc // 2 + 3) // 4
        pw = m["pw"]
        nch = pw.shape[0]
        s = np.arange(P)
        u = np.minimum((s // PAIRW)[None, :] + 4 * np.arange(n_pt)[:, None],
                       tpc // 2 - 1)
        tgt = pw[:, u] + (s % PAIRW)[None, None, :]      # [nch, n_pt, P]
        part = out.transpose(0, 2, 1, 3).reshape(nch * n_pt * P, D)
        tgt = tgt.transpose(0, 1, 2).reshape(-1)
        ok = tgt < G
        np.add.at(rep, tgt[ok], part[ok])
    return rep


_LAST_RUN = {}


def kernel(features, segment_ids, num_segments, weight_matrix):
    from concourse.bass_utils import run_bass_kernel_spmd
    _ensure_ntff_hook()

    G = int(num_segments)
    D = features.shape[1]
    prep = host_prep(features, segment_ids, num_segments, weight_matrix,
                     tpc=32, strict=False)
    tpc = 32
    if prep is None:
        tpc = 16
        prep = host_prep(features, segment_ids, num_segments, weight_matrix,
                         tpc=16, strict=True)
    nch, in_maps, meta, cnt = prep
    nc = build_program(nch, tpc)
    trace = bool(int(os.environ.get("BASS_KERNEL_TRACE", "0")))
    kw = {}
    if trace:
        kw["trace"] = True
        kw["tmpdir"] = os.environ.get("BASS_KERNEL_TRACE_DIR") or None
    res = run_bass_kernel_spmd(nc, in_maps, core_ids=list(range(NCORES)), **kw)
    _LAST_RUN["exec_time_ns"] = res.exec_time_ns
    _LAST_RUN["res"] = res
    return assemble(res.results, meta, G, D, cnt)



# revision 39
# speedup vs baseline: 1.2555x; 1.0191x over previous
"""Trainium2 Bass kernel for nn_AttentionModule (segment_reduce).

Computation (per reference):
    wx   = features @ W
    s_g  = segment_sum(wx);  cnt_g = segment counts
    ctx  = tanh(s_g / max(cnt,1))            [G, D]
    score_n = <f_n, ctx[seg_n]>
    rep_g = segment_sum(score_n * f_n)       [G, D]

Key identity: segment_sum(F @ W) = segment_sum(F) @ W, so pass A is a pure
segment-sum.  Device algorithm (SPMD over 8 cores, segment-aligned shard
per core, one uniform compiled program; all data-dependence flows through
input tensors, never instruction addresses):

  per chunk of tpc*128 new nodes (+1 boundary re-read tile):
    - host-built one-hot oh01 (node x 32-wide pair window); segment-sum
      via PE matmuls into 32-aligned psum slot ranges; slot->window merge
      matmul with a host-built m01 that is pre-scaled by 1/count and maps
      each slot to 32-row QUARTER-window rows (32q + seg - wq[q])
    - ctx = tanh(merged @ W), computed as four M=32 matmuls so quarter q
      lands at psum partitions 0:32 -> ctxq [32, 4, d]
    - transposed one-hot ohT[j, n] = (relrow[n] == j) in one DVE
      tensor_scalar(is_equal) against a per-partition iota; relrow holds
      quarter-local ids so ohT is only [32, nodes] (DMA broadcast to 32
      partitions instead of 128)
    - gather ctxg = ohT.T @ ctxq[quarter(t)] (PE, K=32); most gather
      groups are copied psum->sbuf bf16 on ACT so the DVE multiply
      f * ctxg runs in 2x mode; scores = rowsum via fold/fold2/reduce
    - rep via (oh01 * scores) matmul + slot layout, bf16 output DMA
  host: numpy pre/post-processing (index metadata, shard assembly).
"""

import os
import sys
import math
from functools import lru_cache

for _p in ("/opt/trn_rl_repo", "/root/.axon_site/_ro/trn_rl_repo"):
    if os.path.isdir(_p) and _p not in sys.path:
        sys.path.insert(0, _p)

import numpy as np
import ml_dtypes

BF16 = ml_dtypes.bfloat16

P = 128          # partitions
TPC = 16         # new 128-node tiles per chunk
CHUNK = TPC * P  # 2048 new nodes per chunk
TILES = TPC + 1  # + boundary tile (tile 0)
NODES = TILES * P  # nodes touched per chunk (2176)
WINW = 128       # chunk segment-window width
PAIRW = 32       # pair segment-window width
NCORES = 8
MASK = -1000.0


def _ensure_ntff_hook():
    """Register the axon NTFF profile hook if the boot couldn't (antenv stub)."""
    import types
    try:
        import antenv  # noqa
    except ImportError:
        return
    if "antenv.axon_hooks" in sys.modules:
        return
    hooks = types.ModuleType("antenv.axon_hooks")
    holder = [None]
    hooks.set_axon_ntff_profile_hook = lambda h: holder.__setitem__(0, h)
    hooks.get_axon_ntff_profile_hook = lambda: holder[0]
    sys.modules["antenv.axon_hooks"] = hooks
    import antenv
    antenv.axon_hooks = hooks
    try:
        from trn_agent_boot.trn_boot import _ntff_profile_via_ctypes
        so = "/opt/axon/libaxon_pjrt.so"
        if os.path.exists(so):
            hooks.set_axon_ntff_profile_hook(_ntff_profile_via_ctypes(so))
    except Exception:
        pass


@lru_cache(maxsize=4)
def build_program(nch: int, tpc: int = TPC):
    """Build + compile the uniform per-core Bass program.

    Returns (nc, io_names) where io_names lists the dram tensor names.
    """
    import concourse.bass as bass  # noqa
    import concourse.mybir as mybir
    from concourse import bacc, tile

    tiles = tpc + 1
    nodes = tiles * P
    n_pairs = tpc // 2
    n_pt = (n_pairs + 3) // 4          # psum partial tiles (4 pairs each)
    n_g4 = (tiles + 3) // 4            # ctx-gather groups of 4 tiles
    tq = tpc // 4                      # tiles per quarter-window
    f32 = mybir.dt.float32
    bf16 = mybir.dt.bfloat16

    nc = bacc.Bacc("TRN2", target_bir_lowering=False, debug=False,
                   num_devices=NCORES)

    # ---- DRAM I/O ----
    F_d = nc.dram_tensor("f_in", [nch, P, tiles, P], bf16,
                         kind="ExternalInput").ap()
    oh01_d = nc.dram_tensor("oh01", [nch, P, tiles, PAIRW], bf16,
                            kind="ExternalInput").ap()
    relrow_d = nc.dram_tensor("relrow", [nch, 1, nodes], bf16,
                              kind="ExternalInput").ap()
    m01_d = nc.dram_tensor("m01", [nch, P, n_pt, WINW], bf16,
                           kind="ExternalInput").ap()
    piota_d = nc.dram_tensor("piota", [P, 1], mybir.dt.float32,
                             kind="ExternalInput").ap()
    W_d = nc.dram_tensor("w_in", [P, P], bf16, kind="ExternalInput").ap()
    rep_d = nc.dram_tensor("rep_out", [nch, P, n_pt, P], bf16,
                           kind="ExternalOutput").ap()

    AluOp = mybir.AluOpType
    Act = mybir.ActivationFunctionType

    with tile.TileContext(nc) as tc:
        with tc.tile_pool(name="const", bufs=1) as cpool, \
             tc.tile_pool(name="fpool", bufs=4) as fpool, \
             tc.tile_pool(name="small", bufs=4) as spool, \
             tc.tile_pool(name="big", bufs=4) as bpool, \
             tc.tile_pool(name="ps_ctx", bufs=2, space="PSUM") as pss, \
             tc.tile_pool(name="ps_acc", bufs=2, space="PSUM") as psa, \
             tc.tile_pool(name="ps_big", bufs=2, space="PSUM") as psb:

            piota_t = cpool.tile([P, 1], mybir.dt.float32)
            w_t = cpool.tile([P, P], bf16)
            nc.sync.dma_start(piota_t[:], piota_d[:])
            nc.sync.dma_start(w_t[:], W_d[:])

            for k in range(nch):
                # ---- loads ----
                f_t = fpool.tile([P, tiles, P], bf16, tag="f")
                nc.sync.dma_start(f_t[:], F_d[k])

                m01_t = spool.tile([P, n_pt, WINW], bf16, tag="m01")
                nc.sync.dma_start(m01_t[:], m01_d[k])

                # ---- one-hot [node, pair-window] (host-built) ----
                oh_t = spool.tile([P, tiles, PAIRW], bf16, tag="oh")
                nc.sync.dma_start(oh_t[:], oh01_d[k])


                def tile_order():
                    # first MM into each psum partial tile must be a fresh
                    # (start=True) write; boundary tile 0 accumulates into
                    # tile 1's slot range so it must come after tile 1.
                    order = [1, 0] + list(range(2, tiles))
                    return order

                def slot_base(t):
                    u = (t - 1) // 2 if t >= 1 else 0
                    return (u % 4) * PAIRW, u // 4

                # ---- step1: segment sums into slots ----
                order = tile_order()
                # region (pair) u: writers in order; first gets start=True,
                # last gets stop=True.  boundary tile 0 shares region 0.
                region_of = {t: ((t - 1) // 2 if t >= 1 else 0) for t in order}
                first_of_region = {}
                last_of_region = {}
                for t in order:
                    u = region_of[t]
                    first_of_region.setdefault(u, t)
                    last_of_region[u] = t
                ps_s = psa.tile([P, n_pt, P], f32, tag="ps_s",
                                name=f"ps_s_{k}")
                for t in order:
                    base, pt = slot_base(t)
                    u = region_of[t]
                    nc.tensor.matmul(
                        ps_s[base:base + PAIRW, pt, :],
                        oh_t[:, t, :], f_t[:, t, :],
                        start=(first_of_region[u] == t),
                        stop=(last_of_region[u] == t),
                        tile_position=(0, base))

                # ---- merge to sumT [D, w] ----
                ps_sumT = pss.tile([P, WINW], f32, tag="ctxps")
                s_sb = spool.tile([P, n_pt, P], bf16, tag="s_sb")
                nc.scalar.copy(s_sb[:], ps_s[:])
                for pt in range(n_pt):
                    nc.tensor.matmul(ps_sumT[:], s_sb[:, pt, :], m01_t[:, pt, :],
                                     start=(pt == 0), stop=(pt == n_pt - 1))

                # ---- ctx: pre[q] = (recip*sum)[32q:32(q+1)] @ W, quarter-
                # aligned at partitions 0:32 so the gather contracts over
                # K=32 (recip is folded into m01 on the host) ----
                sumT_sb = spool.tile([P, WINW], bf16, tag="sumT_sb")
                nc.scalar.copy(sumT_sb[:], ps_sumT[:])
                ps_preq = pss.tile([PAIRW, 4, P], f32, tag="ctxps")
                for q in range(4):
                    nc.tensor.matmul(ps_preq[:, q, :],
                                     sumT_sb[:, q * PAIRW:(q + 1) * PAIRW],
                                     w_t[:], start=True, stop=True)
                ctxq_t = spool.tile([PAIRW, 4, P], bf16, tag="ctxq")
                nc.scalar.activation(ctxq_t[:], ps_preq[:], Act.Tanh)

                # ---- ohT[j, n] = (relrow[n] == j), quarter-local ----
                relb_t = bpool.tile([PAIRW, nodes], bf16, tag="relb")
                nc.sync.dma_start(relb_t[:],
                                  relrow_d[k].broadcast_to([PAIRW, nodes]))
                ohT_t = bpool.tile([PAIRW, nodes], bf16, tag="ohT")
                nc.vector.tensor_scalar(out=ohT_t[:], in0=relb_t[:],
                                        scalar1=piota_t[0:PAIRW, :],
                                        scalar2=None,
                                        op0=AluOp.is_equal)

                # ---- 3a: CtxG groups + scores ----
                def quarter_of(t):
                    return 0 if t == 0 else min((t - 1) // tq, 3)

                prod_sb = bpool.tile([P, tiles, P], bf16, tag="prod")
                n_act = 9          # gather groups routed psum->sbuf via ACT
                for g in range(n_g4):
                    t0g = g * 4
                    ng = min(4, tiles - t0g)
                    ps_cg = psb.tile([P, 512], f32, tag="bigps")
                    for i in range(ng):
                        t = t0g + i
                        nc.tensor.matmul(
                            ps_cg[:, i * P:(i + 1) * P],
                            ohT_t[:, t * P:(t + 1) * P],
                            ctxq_t[:, quarter_of(t), :],
                            start=(i == 0), stop=(i == ng - 1))
                    if g < n_act:
                        # ACT copies psum->sbuf bf16 so the DVE multiply
                        # runs in 2x mode (all-2-byte operands)
                        cg_sb = spool.tile([P, 512], bf16, tag="cg_sb")
                        nc.scalar.copy(cg_sb[:, :ng * P], ps_cg[:, :ng * P])
                        nc.vector.tensor_tensor(
                            out=prod_sb[:, t0g:t0g + ng, :],
                            in0=f_t[:, t0g:t0g + ng, :],
                            in1=cg_sb[:, :ng * P].rearrange(
                                "p (t d) -> p t d", d=P),
                            op=AluOp.mult)
                    else:
                        nc.vector.tensor_tensor(
                            out=prod_sb[:, t0g:t0g + ng, :],
                            in0=f_t[:, t0g:t0g + ng, :],
                            in1=ps_cg[:, :ng * P].rearrange(
                                "p (t d) -> p t d", d=P),
                            op=AluOp.mult)
                fold_sb = spool.tile([P, tiles, P // 2], bf16, tag="fold")
                nc.vector.tensor_tensor(
                    out=fold_sb[:], in0=prod_sb[:, :, :P // 2],
                    in1=prod_sb[:, :, P // 2:], op=AluOp.add)
                fold2_sb = spool.tile([P, tiles, P // 4], bf16, tag="fold2")
                nc.vector.tensor_tensor(
                    out=fold2_sb[:], in0=fold_sb[:, :, :P // 4],
                    in1=fold_sb[:, :, P // 4:], op=AluOp.add)
                scores_b = spool.tile([P, tiles], bf16, tag="scores_b")
                with nc.allow_low_precision(
                        reason="scores reduce: bf16 out is within budget"):
                    nc.vector.tensor_reduce(out=scores_b[:], in_=fold2_sb[:],
                                            axis=mybir.AxisListType.X,
                                            op=AluOp.add)

                # ---- 3b: rep sums ----
                ohsc_t = spool.tile([P, tiles, PAIRW], bf16, tag="ohsc")
                nc.vector.tensor_tensor(
                    out=ohsc_t[:], in0=oh_t[:],
                    in1=scores_b[:].unsqueeze(2).broadcast_to(
                        [P, tiles, PAIRW]),
                    op=AluOp.mult)
                ps_r = psa.tile([P, n_pt, P], f32, tag="ps_r",
                                name=f"ps_r_{k}")
                for t in order:
                    base, pt = slot_base(t)
                    u = region_of[t]
                    nc.tensor.matmul(
                        ps_r[base:base + PAIRW, pt, :],
                        ohsc_t[:, t, :], f_t[:, t, :],
                        start=(first_of_region[u] == t),
                        stop=(last_of_region[u] == t),
                        tile_position=(0, base))
                r_sb = spool.tile([P, n_pt, P], bf16, tag="r_sb")
                nc.scalar.copy(r_sb[:], ps_r[:])
                nc.sync.dma_start(rep_d[k], r_sb[:])

    nc.compile()
    return nc


def host_prep(features, segment_ids, num_segments, weight_matrix, tpc=TPC,
              strict=True):
    """Numpy preprocessing. Returns (nch, in_maps, meta, cnt) or None if the
    geometry (window spans) doesn't fit for this tpc."""
    N, D = features.shape
    G = int(num_segments)
    seg = np.asarray(segment_ids).astype(np.int64)
    feats = np.asarray(features, dtype=np.float32)
    W = np.asarray(weight_matrix, dtype=np.float32)

    chunk = tpc * P
    tiles = tpc + 1
    nodes = tiles * P
    n_pt = (tpc // 2 + 3) // 4

    bnd = np.searchsorted(seg, np.arange(G + 1))
    cnt = np.diff(bnd)
    if cnt.max() > P:
        assert not strict, f"segment with {cnt.max()} nodes > {P}"
        return None
    recip_full = np.where(cnt > 0, 1.0 / np.maximum(cnt, 1), 0.0).astype(np.float32)

    cuts = [0]
    for c in range(1, NCORES):
        gidx = min(int(np.searchsorted(bnd, round(c * N / NCORES))), G)
        cuts.append(int(bnd[gidx]))
    cuts.append(N)
    counts = [cuts[c + 1] - cuts[c] for c in range(NCORES)]
    nch = max(1, math.ceil(max(counts) / chunk))

    in_maps = []
    meta = []
    for c in range(NCORES):
        n0, n1 = cuts[c], cuts[c + 1]
        Nc = n1 - n0
        segl = seg[n0:n1]

        f_pad = np.zeros((P + nch * chunk, D), BF16)
        f_pad[P:P + Nc] = feats[n0:n1].astype(BF16)
        f_in = np.lib.stride_tricks.sliding_window_view(
            f_pad, (nodes, D))[::chunk, 0][:nch]
        f_in = np.ascontiguousarray(
            f_in.reshape(nch, tiles, P, D).transpose(0, 2, 1, 3))

        v = np.arange(Nc)
        chunk_of = v // chunk
        g_lo, g_hi = int(segl[0]), int(segl[-1]) + 1
        own = (bnd[np.arange(g_lo, g_hi) + 1] - 1 - n0) // chunk
        own_of_node = own[segl - g_lo]
        valid = own_of_node == chunk_of

        pw = np.full((nch, tpc // 2), 0, np.int64)
        for k in range(nch):
            for u in range(tpc // 2):
                i = k * chunk + u * 2 * P
                pw[k, u] = segl[min(i, Nc - 1)]

        # quarter-window bases: wq[k, q] = min valid seg in (chunk, quarter)
        tq = tpc // 4
        qn = ((v % chunk) // P) // tq
        wq = np.full((nch, 4), 1 << 40, np.int64)
        np.minimum.at(wq, (chunk_of[valid], qn[valid]), segl[valid])

        relp = np.where(valid, segl - pw[chunk_of, ((v % chunk) // P) // 2],
                        MASK).astype(np.float32)
        relq = np.where(valid, segl - wq[chunk_of, qn], MASK
                        ).astype(np.float32)

        rel32 = np.full((nch, P, tiles), MASK, np.float32)
        brow = np.full((nch, tiles * P), MASK, np.float32)
        pad = np.full(nch * chunk - Nc, MASK, np.float32)
        rp = np.concatenate([relp, pad]).reshape(nch, tpc, P)
        rw = np.concatenate([relq, pad]).reshape(nch, tpc, P)
        rel32[:, :, 1:] = rp.transpose(0, 2, 1)
        brow[:, P:] = rw.reshape(nch, -1)

        for k in range(1, nch):
            lo = k * chunk - P
            if lo >= Nc:
                continue
            hi = min(k * chunk, Nc)
            idx = np.arange(lo, hi)
            bvalid = own_of_node[idx] == k
            if bvalid.any():
                wq[k, 0] = min(wq[k, 0], int(segl[idx][bvalid].min()))
            br = np.where(bvalid, segl[idx] - wq[k, 0], MASK
                          ).astype(np.float32)
            rel32[k, :hi - lo, 0] = br
            brow[k, :hi - lo] = br

        # boundary nodes share quarter 0: re-derive new-node relq of q0
        # against the (possibly lowered) wq[:, 0]
        q0 = valid & (qn == 0)
        relq2 = segl[q0] - wq[chunk_of[q0], 0]
        rw2 = np.full(nch * chunk, MASK, np.float32)
        rw2[:Nc][q0] = relq2
        rw2v = rw2.reshape(nch, tpc, P)
        mask_q0 = np.zeros((nch, tpc, P), bool)
        mask_q0[:, :tq, :] = True
        brow_v = brow[:, P:].reshape(nch, tpc, P)
        brow_v[mask_q0] = rw2v[mask_q0]

        # geometry checks (fall back to smaller tpc on overflow)
        rel_ok = rel32[rel32 > MASK / 2]
        brow_ok = brow[brow > MASK / 2]
        bad = bool(rel_ok.size and (rel_ok.min() < 0
                                    or rel_ok.max() >= PAIRW)) or \
            bool(brow_ok.size and (brow_ok.min() < 0
                                   or brow_ok.max() >= PAIRW))
        if bad:
            assert not strict, "window overflow"
            return None

        relrow = brow.reshape(nch, 1, tiles * P).astype(BF16)

        # merge matrix: slot (pair u, j) -> window row 32q + (seg - wq[q]),
        # pre-scaled by 1/count so ctx = tanh(sum @ W) directly
        m01 = np.zeros((nch, P, n_pt, WINW), np.float32)
        sl = np.arange(P)
        for pt in range(n_pt):
            u = np.minimum(4 * pt + sl // PAIRW, tpc // 2 - 1)
            sseg = pw[:, u] + (sl % PAIRW)[None, :]       # [nch, P]
            okseg = sseg < G
            rec = np.where(okseg, recip_full[np.minimum(sseg, G - 1)], 0.0)
            for q in range(4):
                r = sseg - wq[:, q][:, None]              # [nch, P]
                hit = (r >= 0) & (r < PAIRW) & okseg
                kk, ss = np.nonzero(hit)
                m01[kk, ss, pt, PAIRW * q + r[kk, ss]] = rec[kk, ss]
        m01 = m01.astype(BF16)

        oh01 = (rel32[..., None] ==
                np.arange(PAIRW, dtype=np.float32)).astype(BF16)

        in_maps.append({
            "f_in": f_in,
            "oh01": oh01,
            "relrow": relrow,
            "m01": m01,
            "w_in": W.astype(BF16),
            "piota": np.arange(P, dtype=np.float32)[:, None],
        })
        meta.append({"n0": n0, "n1": n1, "g_lo": g_lo, "g_hi": g_hi,
                     "own": own, "wq": wq, "pw": pw, "tpc": tpc})
    return nch, in_maps, meta, cnt


def assemble(results, meta, G, D, cnt=None):
    rep = np.zeros((G, D), np.float32)
    for c in range(NCORES):
        out = np.asarray(results[c]["rep_out"], dtype=np.float32)
        m = meta[c]
        tpc = m["tpc"]
        n_pt = (tpc // 2 + 3) // 4
        pw = m["pw"]
        nch = pw.shape[0]
        s = np.arange(P)
        u = np.minimum((s // PAIRW)[None, :] + 4 * np.arange(n_pt)[:, None],
                       tpc // 2 - 1)
        tgt = pw[:, u] + (s % PAIRW)[None, None, :]      # [nch, n_pt, P]
        part = out.transpose(0, 2, 1, 3).reshape(nch * n_pt * P, D)
        tgt = tgt.transpose(0, 1, 2).reshape(-1)
        ok = tgt < G
        np.add.at(rep, tgt[ok], part[ok])
    return rep


_LAST_RUN = {}


def kernel(features, segment_ids, num_segments, weight_matrix):
    from concourse.bass_utils import run_bass_kernel_spmd
    _ensure_ntff_hook()

    G = int(num_segments)
    D = features.shape[1]
    prep = host_prep(features, segment_ids, num_segments, weight_matrix,
                     tpc=32, strict=False)
    tpc = 32
    if prep is None:
        tpc = 16
        prep = host_prep(features, segment_ids, num_segments, weight_matrix,
                         tpc=16, strict=True)
    nch, in_maps, meta, cnt = prep
    nc = build_program(nch, tpc)
    trace = bool(int(os.environ.get("BASS_KERNEL_TRACE", "0")))
    kw = {}
    if trace:
        kw["trace"] = True
        kw["tmpdir"] = os.environ.get("BASS_KERNEL_TRACE_DIR") or None
    res = run_bass_kernel_spmd(nc, in_maps, core_ids=list(range(NCORES)), **kw)
    _LAST_RUN["exec_time_ns"] = res.exec_time_ns
    _LAST_RUN["res"] = res
    return assemble(res.results, meta, G, D, cnt)

